# revision 5
# baseline (speedup 1.0000x reference)
"""Trainium2 Bass kernel for nn_AttentionBlockWithSelfAttention.

Computes (per reference.py):
    c   = relu(BN(conv1x1(g, wg)) + BN(conv1x1(x, wx)))        # [B, 64, N]
    q/k/v convs, energy = q^T k, attn = softmax(energy)
    sa  = v @ attn^T ; attended = gamma*sa + c
    psi = sigmoid(BN(conv1x1(attended, psi_w)))                # [B, 1, N]
    out = x * psi

Sharding: 8 cores = (batch b = core//2, row-half h = core%2). Host rolls
spatial columns by 2048*h so each core runs the identical SPMD program on
columns [0:2048] = its attention rows; attention reductions over all m are
permutation-invariant so the roll is exact.

Algebraic reductions done on host (all O(C^2) on weights):
  - BN folded into conv weights/biases.
  - q/k biases: softmax rows are invariant to per-row constants, so
    energy' = (q no-bias)^T (k with-bias) gives identical softmax.
  - `attended` only feeds the 1-channel psi conv, so per-channel sa_out is
    never needed: the v/sa stage reduces to 2 rows computed in one
    accumulating matmul: row0 = softmax denominator sums[n],
    row1 = A[n] = sum_c uA[c] * sa_unnorm[c, n] (uA = gamma*s_psi*psi_w).
  - psi_pre[n] = A[n]/sums[n] + C[n] + t0, with C[n] = uC^T cin[:, n].
"""

import sys

if "/opt/trn_rl_repo" not in sys.path:
    sys.path.insert(0, "/opt/trn_rl_repo")

import numpy as np

import concourse.bass as bass
import concourse.tile as tile
from concourse import bacc, mybir
from concourse.bass_utils import run_bass_kernel_spmd

EPS = 1e-5
B, FG, FI, FQ, H, W = 4, 128, 64, 8, 64, 64
N = H * W            # 4096
NCORES = 8
NL = N // 2          # 2048 local attention rows per core
MT = N // 128        # 32 m-tiles of 128
NB = 1024            # n-block for energy/exp tiles
F32 = mybir.dt.float32
F32R = mybir.dt.float32r
AFT = mybir.ActivationFunctionType
ALU = mybir.AluOpType

_CACHE = {}


def _emit(nc, tc, d):
    """Emit the single-core SPMD program. d: dict of DRAM tensor handles."""
    import contextlib

    ctx = contextlib.ExitStack()
    with ctx:
        wp = ctx.enter_context(tc.tile_pool(name="wp", bufs=1))
        bp = ctx.enter_context(tc.tile_pool(name="bp", bufs=1))
        ep = ctx.enter_context(tc.tile_pool(name="ep", bufs=3))
        pe_ps = ctx.enter_context(tc.tile_pool(name="pe_ps", bufs=2, space="PSUM"))
        acc_ps = ctx.enter_context(tc.tile_pool(name="acc_ps", bufs=1, space="PSUM"))

        # ---- small weights to SBUF -----------------------------------
        wgT = wp.tile([FG, FI + 1], F32)
        nc.sync.dma_start(wgT[:], d["wgT"][:])
        wxT = wp.tile([FG, FI + 1], F32)
        nc.sync.dma_start(wxT[:], d["wxT"][:])
        cb = wp.tile([FI + 1, 1], F32)
        nc.sync.dma_start(cb[:], d["cb"][:])
        qkT = wp.tile([FI + 1, 2 * FQ], F32)
        nc.sync.dma_start(qkT[:], d["qkT"][:])
        vw2 = wp.tile([FI + 1, 2], F32)
        nc.sync.dma_start(vw2[:], d["vw2"][:])
        uC = wp.tile([FI + 1, 1], F32)
        nc.sync.dma_start(uC[:], d["uC"][:])
        nt0 = wp.tile([1, 1], F32)
        nc.sync.dma_start(nt0[:], d["nt0"][:])

        # f32r-rounded weight copies (PE f32r operands must be produced
        # by a rounding op, not raw DMA)
        qkT_r = wp.tile([FI + 1, 2 * FQ], F32R)
        nc.vector.tensor_copy(qkT_r[:], qkT[:])
        vw2_r = wp.tile([FI + 1, 2], F32R)
        nc.vector.tensor_copy(vw2_r[:], vw2[:])
        uC_r = wp.tile([FI + 1, 1], F32R)
        nc.vector.tensor_copy(uC_r[:], uC[:])
        onesf = wp.tile([1, 128], F32)
        nc.gpsimd.memset(onesf[:], 1.0)
        ones_r = wp.tile([1, 128], F32R)
        nc.vector.tensor_copy(ones_r[:], onesf[:])

        # ---- big inputs ----------------------------------------------
        g_sb = bp.tile([FG, N], F32)
        x_sb = bp.tile([FG, N], F32)
        for ch in range(2):
            s = bass.ts(ch, N // 2)
            nc.sync.dma_start(g_sb[:, s], d["g"][:, s])
            nc.sync.dma_start(x_sb[:, s], d["x"][:, s])

        cin = bp.tile([FI + 1, N], F32R)     # rows 0..63 data, row 64 ones
        qk2 = bp.tile([FQ, 2, N], F32R)      # [:,0,:]=q(no bias) [:,1,:]=k(+bias)
        vT = bp.tile([128, MT, 2], F32R)     # col0 -> sums, col1 -> A weights
        cn_sb = bp.tile([1, NL], F32)        # C row
        out_sb = bp.tile([FG, NL], F32)

        # ---- cin = relu(wg@g + wx@x + cb), row 64 = relu(0+1) = 1 ----
        for nt in range(N // 512):
            s = bass.ts(nt, 512)
            ps = pe_ps.tile([FI + 1, 512], F32, tag="et")
            nc.tensor.matmul(ps[:], wgT[:], g_sb[:, s], start=True, stop=False)
            nc.tensor.matmul(ps[:], wxT[:], x_sb[:, s], start=False, stop=True)
            nc.vector.tensor_scalar(cin[:, s], ps[:], cb[:], 0.0, ALU.add, ALU.max)

        # ---- q / k  (q = qw^T cin, k = kw^T cin + kb via ones row) ---
        for nt in range(N // 512):
            s = bass.ts(nt, 512)
            ps = pe_ps.tile([FQ, 2 * 512], F32, tag="et")
            nc.tensor.matmul(ps[:, 0:512], qkT_r[:, 0:FQ], cin[:, s])
            nc.tensor.matmul(ps[:, 512:1024], qkT_r[:, FQ : 2 * FQ], cin[:, s])
            nc.vector.tensor_copy(
                qk2[:, :, s],
                ps[:].rearrange("p (t n) -> p t n", t=2),
            )

        # ---- vT[:, mt, 0:2]: per-m-tile [sums-maker | A-maker] -------
        for grp in range(MT // 8):
            ps = pe_ps.tile([128, 16], F32, tag="et")
            for j in range(8):
                mt = grp * 8 + j
                nc.tensor.matmul(
                    ps[:, 2 * j : 2 * j + 2],
                    cin[:, bass.ts(mt, 128)],
                    vw2_r[:],
                )
            nc.vector.tensor_copy(
                vT[:, bass.ts(grp, 8), :],
                ps[:].rearrange("p (t n) -> p t n", t=8),
            )

        # ---- C row: uC^T cin over local columns ----------------------
        for j in range(NL // 512):
            s = bass.ts(j, 512)
            ps = pe_ps.tile([1, 512], F32, tag="et")
            nc.tensor.matmul(ps[:], uC_r[:], cin[:, s])
            nc.vector.tensor_copy(cn_sb[:, s], ps[:])

        # ---- main attention loop -------------------------------------
        # acc row0 = sums[n], row1 = A[n]; 4 banks, one per 512-slice
        accs = [acc_ps.tile([2, 512], F32, name=f"acc{_j}") for _j in range(NL // 512)]
        for mt in range(MT):
            kap = qk2[:, 1, bass.ts(mt, 128)]
            for hb in range(NL // NB):
                ps = pe_ps.tile([128, NB], F32, tag="et")
                for s in range(NB // 512):
                    nc.tensor.matmul(
                        ps[:, bass.ts(s, 512)],
                        kap,
                        qk2[:, 0, bass.ds(hb * NB + s * 512, 512)],
                    )
                ex = ep.tile([128, NB], F32R)
                nc.scalar.activation(ex[:], ps[:], AFT.Exp)
                for s in range(NB // 512):
                    nc.tensor.matmul(
                        accs[hb * (NB // 512) + s][:],
                        vT[:, mt, :],
                        ex[:, bass.ts(s, 512)],
                        start=(mt == 0),
                        stop=(mt == MT - 1),
                    )

        # ---- epilogue: psi = sigmoid(A/sums + C + t0); out = x * psi -
        # engine APs must start at partition 0, so stage [2, NL] to SBUF
        # and DMA row 1 (A) down to a partition-0 tile.
        sa2 = wp.tile([2, NL], F32)
        for j in range(NL // 512):
            nc.vector.tensor_copy(sa2[:, bass.ts(j, 512)], accs[j][:])
        a_sb = wp.tile([1, NL], F32)
        nc.sync.dma_start(a_sb[:], sa2[1:2, :])
        rec = wp.tile([1, NL], F32)
        t1 = wp.tile([1, NL], F32)
        t2 = wp.tile([1, NL], F32)
        uu = wp.tile([1, NL], F32)
        psi_r = wp.tile([1, NL], F32R)
        nc.vector.reciprocal_approx_fast(rec[:], sa2[0:1, :])
        nc.vector.tensor_mul(t1[:], a_sb[:], rec[:])
        nc.vector.tensor_add(t2[:], t1[:], cn_sb[:])
        # exp(-(psi_pre)) = exp(-t2 - t0): scale=-1, bias=-t0 (nt0)
        nc.scalar.activation(uu[:], t2[:], AFT.Exp, bias=nt0[:], scale=-1.0)
        nc.vector.tensor_scalar(uu[:], uu[:], 1.0, None, ALU.add)
        nc.vector.reciprocal_approx_fast(t1[:], uu[:])
        nc.vector.tensor_copy(psi_r[:], t1[:])
        for j in range(NL // 512):
            s = bass.ts(j, 512)
            bc = pe_ps.tile([128, 512], F32, tag="et")
            nc.tensor.matmul(bc[:], ones_r[:], psi_r[:, s])
            nc.vector.tensor_mul(out_sb[:, s], x_sb[:, s], bc[:])
            nc.sync.dma_start(d["out"][:, s], out_sb[:, s])


def _build():
    nc = bacc.Bacc(
        "TRN2", target_bir_lowering=False, debug=False, num_devices=NCORES
    )
    d = {}
    d["g"] = nc.declare_dram_parameter("g", [FG, N], F32, isOutput=False)
    d["x"] = nc.declare_dram_parameter("x", [FG, N], F32, isOutput=False)
    d["wgT"] = nc.declare_dram_parameter("wgT", [FG, FI + 1], F32, isOutput=False)
    d["wxT"] = nc.declare_dram_parameter("wxT", [FG, FI + 1], F32, isOutput=False)
    d["cb"] = nc.declare_dram_parameter("cb", [FI + 1, 1], F32, isOutput=False)
    d["qkT"] = nc.declare_dram_parameter("qkT", [FI + 1, 2 * FQ], F32, isOutput=False)
    d["vw2"] = nc.declare_dram_parameter("vw2", [FI + 1, 2], F32, isOutput=False)
    d["uC"] = nc.declare_dram_parameter("uC", [FI + 1, 1], F32, isOutput=False)
    d["nt0"] = nc.declare_dram_parameter("nt0", [1, 1], F32, isOutput=False)
    d["out"] = nc.declare_dram_parameter("out", [FG, NL], F32, isOutput=True)

    with tile.TileContext(nc) as tc:
        _emit(nc, tc, d)
    nc.compile()
    return nc


def _host_params(inputs):
    """Fold BN / biases into the small weight tensors (see module docstring)."""
    f64 = {k: np.asarray(v, np.float64) for k, v in inputs.items()}
    sg = f64["bng_w"] / np.sqrt(f64["bng_v"] + EPS)
    sx = f64["bnx_w"] / np.sqrt(f64["bnx_v"] + EPS)
    wg = f64["wg_w"] * sg[:, None]          # [64, 128]
    wx = f64["wx_w"] * sx[:, None]
    cbv = (f64["wg_b"] - f64["bng_m"]) * sg + f64["bng_b"] + (
        f64["wx_b"] - f64["bnx_m"]
    ) * sx + f64["bnx_b"]                    # [64]

    wgT = np.zeros((FG, FI + 1), np.float32)
    wgT[:, :FI] = wg.T
    wxT = np.zeros((FG, FI + 1), np.float32)
    wxT[:, :FI] = wx.T
    cb = np.zeros((FI + 1, 1), np.float32)
    cb[:FI, 0] = cbv
    cb[FI, 0] = 1.0                          # makes cin row 64 = relu(0+1) = 1

    qkT = np.zeros((FI + 1, 2 * FQ), np.float32)
    qkT[:FI, :FQ] = f64["q_w"].T             # q, no bias
    qkT[:FI, FQ:] = f64["k_w"].T             # k
    qkT[FI, FQ:] = f64["k_b"]                # k bias via cin ones row

    s_psi = (f64["bnp_w"] / np.sqrt(f64["bnp_v"] + EPS))[0]
    gamma = f64["sa_gamma"][0]
    uA = gamma * s_psi * f64["psi_w"][0]     # [64]
    uCv = s_psi * f64["psi_w"][0]            # [64]

    # vwT_aug[ci, c]: v^T maker incl. bias row;  vw2 = [ones-maker | vwT_aug@uA]
    vwT_aug = np.zeros((FI + 1, FI), np.float64)
    vwT_aug[:FI, :] = f64["v_w"].T
    vwT_aug[FI, :] = f64["v_b"]
    vw2 = np.zeros((FI + 1, 2), np.float32)
    vw2[FI, 0] = 1.0                         # sums via cin ones row
    vw2[:, 1] = vwT_aug @ uA                 # A-row maker

    uC = np.zeros((FI + 1, 1), np.float32)
    uC[:FI, 0] = uCv
    t0 = s_psi * (f64["psi_b"][0] - f64["bnp_m"][0]) + f64["bnp_b"][0]
    nt0 = np.full((1, 1), -t0, np.float32)

    return dict(wgT=wgT, wxT=wxT, cb=cb, qkT=qkT, vw2=vw2, uC=uC, nt0=nt0)


def kernel(**inputs):
    if "nc" not in _CACHE:
        _CACHE["nc"] = _build()
    nc = _CACHE["nc"]

    params = _host_params(inputs)
    g = np.ascontiguousarray(np.asarray(inputs["g"], np.float32).reshape(B, FG, N))
    x = np.ascontiguousarray(np.asarray(inputs["x"], np.float32).reshape(B, FG, N))

    in_maps = []
    for c in range(NCORES):
        b, h = divmod(c, 2)
        if h == 0:
            gc, xc = g[b], x[b]
        else:
            gc = np.roll(g[b], -NL, axis=1)
            xc = np.roll(x[b], -NL, axis=1)
        m = {"g": np.ascontiguousarray(gc), "x": np.ascontiguousarray(xc)}
        m.update(params)
        in_maps.append(m)

    res = run_bass_kernel_spmd(nc, in_maps, list(range(NCORES)))

    out = np.empty((B, FG, N), np.float32)
    for c in range(NCORES):
        b, h = divmod(c, 2)
        out[b, :, h * NL : (h + 1) * NL] = res.results[c]["out"]
    return out.reshape(B, FG, H, W)
